# revision 11
# baseline (speedup 1.0000x reference)
"""Trainium2 Bass kernel for nn_DecisionSufficientAbstraction (topk_masking).

Reference computation (per batch row b):
    query   = Wq @ latent[b,0] + bq
    keys_n  = Wk @ latent[b,n] + bk
    sim_n   = (query . keys_n) / sqrt(D)
    sal_n   = Ws . latent[b,n] + bs
    score_n = sim_n + sal_n              (masked to -inf where ~token_mask)
    top-64 -> selected_{scores,indices}; gather tokens; masked mean; softmax.

Algebraic fusion used on device: score_n = latent[b,n] . w_b + c_b with
    w_b = (Wk^T (Wq latent[b,0] + bq)) / 16 + Ws
and c_b a per-row constant.  The constant shifts every score of a row equally,
so it changes neither the top-k selection/ordering nor the softmax; it is
dropped entirely (bk/bs never reach the device).  Invalid tokens get -1e38
added, which absorbs the finite score exactly (|score| << ulp(1e38)), so all
invalid scores are identical, matching the reference's -inf tie behavior.

Sharding: pure data parallel, batch 128 -> 8 cores x 16 rows, weights
replicated.  Each core streams its 32MB latent shard once.

Per-row on-chip layout: token n lives at (partition p, chunk s) with
n = p*16 + s; each partition's 16 tokens are contiguous 16KB in HBM, so the
main DMA is 128 descriptors x 16KB per row.
"""

import sys

for _p in ("/opt/trn_rl_repo", "/root/.axon_site/_ro/trn_rl_repo"):
    if _p not in sys.path:
        sys.path.append(_p)

from contextlib import ExitStack

import numpy as np

import concourse.bass as bass
import concourse.tile as tile
from concourse import bacc, mybir
from concourse.bass_utils import run_bass_kernel_spmd
from concourse.masks import make_identity

B, N, D, K = 128, 2048, 256, 64
NCORES = 8
BL = B // NCORES  # 16 rows per core
S = N // 128      # 16 chunks per row (token n = p*16 + s)
F32 = mybir.dt.float32
NEG_BIG = -1.0e38   # added to masked-out scores
NEG_FILL = -3.0e38  # match_replace fill; sorts below masked scores


def build_nc():
    nc = bacc.Bacc("TRN2", target_bir_lowering=False, debug=False,
                   num_devices=NCORES)

    lat = nc.dram_tensor("latent", [BL, N, D], F32, kind="ExternalInput")
    m01 = nc.dram_tensor("mask01", [BL, N], F32, kind="ExternalInput")
    wqT = nc.dram_tensor("wqT", [D, D], F32, kind="ExternalInput")
    wk = nc.dram_tensor("wk", [D, D], F32, kind="ExternalInput")
    ws16 = nc.dram_tensor("ws16", [1, D], F32, kind="ExternalInput")
    bqd = nc.dram_tensor("bqd", [D], F32, kind="ExternalInput")

    out_tok = nc.dram_tensor("out_tokens", [BL * K, D], F32, kind="ExternalOutput")
    out_idx = nc.dram_tensor("out_idx", [BL, K], mybir.dt.int32, kind="ExternalOutput")
    out_maskf = nc.dram_tensor("out_maskf", [BL, K], F32, kind="ExternalOutput")
    out_imp = nc.dram_tensor("out_imp", [BL, K], F32, kind="ExternalOutput")
    out_glob = nc.dram_tensor("out_glob", [BL, D], F32, kind="ExternalOutput")

    AX = mybir.AxisListType
    OP = mybir.AluOpType
    AF = mybir.ActivationFunctionType

    with tile.TileContext(nc) as tc, ExitStack() as ctx:
        const = ctx.enter_context(tc.tile_pool(name="const", bufs=1))
        work = ctx.enter_context(tc.tile_pool(name="work", bufs=2))
        small = ctx.enter_context(tc.tile_pool(name="small", bufs=2))
        psum = ctx.enter_context(tc.tile_pool(name="psum", bufs=1, space="PSUM"))
        psumg = ctx.enter_context(tc.tile_pool(name="psumg", bufs=2, space="PSUM"))

        # ---------- Phase A: constants + per-row combined weight ----------
        ident = const.tile([128, 128], F32)
        make_identity(nc, ident[:])
        ones1 = const.tile([1, 128], F32)
        nc.vector.memset(ones1[:], 1.0)

        wq_sb = const.tile([128, 2 * D], F32)  # wqT d-chunk c at [:, c*D:(c+1)*D]
        nc.sync.dma_start(out=wq_sb[:, 0:D], in_=wqT[0:128, :])
        nc.sync.dma_start(out=wq_sb[:, D:2 * D], in_=wqT[128:256, :])
        wk_sb = const.tile([128, 2 * D], F32)  # wk e-chunk c at [:, c*D:(c+1)*D]
        nc.sync.dma_start(out=wk_sb[:, 0:D], in_=wk[0:128, :])
        nc.sync.dma_start(out=wk_sb[:, D:2 * D], in_=wk[128:256, :])
        ws_sb = const.tile([1, D], F32)
        nc.sync.dma_start(out=ws_sb[:], in_=ws16[:])
        bq_sb = const.tile([128, 2], F32)  # bq e-chunk c in column c
        nc.sync.dma_start(out=bq_sb[:], in_=bqd.ap().rearrange("(c p) -> p c", p=128))

        ego = small.tile([BL, D], F32)
        nc.sync.dma_start(out=ego[:], in_=lat[:, 0, :])

        egoT = small.tile([128, 2 * BL], F32)  # d-chunk c at [:, c*BL:(c+1)*BL]
        for c in range(2):
            pt = psum.tile([128, BL], F32, tag="ptrans")
            nc.tensor.transpose(out=pt[:], in_=ego[:, c * 128:(c + 1) * 128],
                                identity=ident[0:BL, 0:BL])
            nc.vector.tensor_copy(out=egoT[:, c * BL:(c + 1) * BL], in_=pt[:])

        # V^T[e, b] = sum_d wqT[d, e] egoT[d, b]  (+ bq)
        vt = small.tile([128, 2 * BL], F32)  # e-chunk c at [:, c*BL:(c+1)*BL]
        for ec in range(2):
            pv = psum.tile([128, BL], F32, tag="pvt")
            for dc in range(2):
                nc.tensor.matmul(
                    out=pv[:],
                    lhsT=wq_sb[:, dc * D + ec * 128: dc * D + ec * 128 + 128],
                    rhs=egoT[:, dc * BL:(dc + 1) * BL],
                    start=(dc == 0), stop=(dc == 1))
            nc.vector.tensor_scalar(out=vt[:, ec * BL:(ec + 1) * BL], in0=pv[:],
                                    scalar1=bq_sb[:, ec:ec + 1], scalar2=None,
                                    op0=OP.add)

        # U[b, d] = sum_e V^T[e, b] wk[e, d] + 16*Ws[d];  w_comb = U / 16
        pu = psum.tile([BL, D], F32)
        for ec in range(2):
            nc.tensor.matmul(out=pu[:], lhsT=vt[:, ec * BL:(ec + 1) * BL],
                             rhs=wk_sb[:, ec * D:(ec + 1) * D],
                             start=(ec == 0), stop=False)
        nc.tensor.matmul(out=pu[:], lhsT=ones1[:, 0:BL], rhs=ws_sb[:],
                         start=False, stop=True)
        wcomb = small.tile([BL, D], F32)
        nc.vector.tensor_scalar(out=wcomb[:], in0=pu[:], scalar1=1.0 / 16.0,
                                scalar2=None, op0=OP.mult)

        # broadcast each row's w_comb across 128 partitions; PE operands must
        # start at partition 0, so first flatten wcomb onto one partition.
        wrow = small.tile([1, BL * D], F32)
        nc.sync.dma_start(
            out=wrow[0:1, :].rearrange("o (b d) -> o b d", d=D), in_=wcomb[:])
        wb_all = const.tile([128, BL * D], F32)
        for b in range(BL):
            pw = psum.tile([128, D], F32, tag="pw")
            nc.tensor.matmul(out=pw[:], lhsT=ones1[:],
                             rhs=wrow[0:1, b * D:(b + 1) * D],
                             start=True, stop=True)
            nc.vector.tensor_copy(out=wb_all[:, b * D:(b + 1) * D], in_=pw[:])

        # mask in row-major [BL, N] for the per-row valid counts
        m01_all = const.tile([BL, N], F32)
        nc.sync.dma_start(out=m01_all[:], in_=m01[:])
        cnt = small.tile([BL, 1], F32)
        nc.vector.tensor_reduce(out=cnt[:], in_=m01_all[:], axis=AX.X, op=OP.add)
        inv_cnt = small.tile([BL, 1], F32)
        nc.vector.reciprocal(out=inv_cnt[:], in_=cnt[:])
        # flatten to partition 0 so it can scale partition-0 psum rows
        inv_row = const.tile([1, BL], F32)
        nc.sync.dma_start(out=inv_row[:], in_=inv_cnt[:])

        # ---------- Phase B: stream latent, scores + masked sum ----------
        scores_all = const.tile([BL, N], F32)

        for b in range(BL):
            L = work.tile([128, S * D], F32, tag="L")
            nc.sync.dma_start(
                out=L[:].rearrange("p (s d) -> p s d", d=D),
                in_=lat[b].rearrange("(p s) d -> p s d", s=S))
            m_col = work.tile([128, S], F32, tag="mcol")
            nc.sync.dma_start(out=m_col[:], in_=m01[b].rearrange("(p s) -> p s", s=S))

            prod = work.tile([128, S * D], F32, tag="prod")
            nc.vector.tensor_tensor(
                out=prod[:].rearrange("p (s d) -> p s d", d=D),
                in0=L[:].rearrange("p (s d) -> p s d", d=D),
                in1=wb_all[:, b * D:(b + 1) * D]
                    .rearrange("p (s d) -> p s d", s=1).to_broadcast([128, S, D]),
                op=OP.mult)

            scores_col = work.tile([128, S], F32, tag="scol")
            pg_row = psumg.tile([1, D], F32, tag="pgrow")
            for s in range(S):
                scratch = work.tile([128, D], F32, tag="actscratch")
                nc.scalar.activation(out=scratch[:], in_=prod[:, s * D:(s + 1) * D],
                                     func=AF.Copy,
                                     accum_out=scores_col[:, s:s + 1])
                nc.tensor.matmul(out=pg_row[:], lhsT=m_col[:, s:s + 1],
                                 rhs=L[:, s * D:(s + 1) * D],
                                 start=(s == 0), stop=(s == S - 1))
            # masked sum / count for this row, straight to DRAM
            glob_row = work.tile([1, D], F32, tag="globrow")
            nc.scalar.activation(out=glob_row[:], in_=pg_row[:], func=AF.Copy,
                                 scale=inv_row[0:1, b:b + 1])
            nc.sync.dma_start(out=out_glob[b:b + 1, :], in_=glob_row[:])

            # invalid tokens: += -1e38 (absorbs the score exactly)
            mb_col = work.tile([128, S], F32, tag="mbcol")
            nc.vector.tensor_scalar(out=mb_col[:], in0=m_col[:], scalar1=1.0,
                                    scalar2=-NEG_BIG, op0=OP.subtract, op1=OP.mult)
            nc.vector.tensor_tensor(out=scores_col[:], in0=scores_col[:],
                                    in1=mb_col[:], op=OP.add)
            # flatten [128, S] -> row b of scores_all (free offset p*16+s)
            nc.sync.dma_start(
                out=scores_all[b:b + 1, :].rearrange("o (p s) -> o p s", s=S),
                in_=scores_col[:])

        # ---------- Phase C: top-64 per row ----------
        vals = const.tile([BL, K], F32)
        idxu = const.tile([BL, K], mybir.dt.uint32)
        for r in range(K // 8):
            sl = slice(r * 8, r * 8 + 8)
            nc.vector.max(out=vals[:, sl], in_=scores_all[:])
            nc.vector.max_index(out=idxu[:, sl], in_max=vals[:, sl],
                                in_values=scores_all[:])
            nc.vector.match_replace(out=scores_all[:], in_to_replace=vals[:, sl],
                                    in_values=scores_all[:], imm_value=NEG_FILL)

        idx_i32 = const.tile([BL, K], mybir.dt.int32)
        nc.vector.tensor_copy(out=idx_i32[:], in_=idxu[:])
        nc.sync.dma_start(out=out_idx[:], in_=idx_i32[:])

        maskf = small.tile([BL, K], F32)
        nc.vector.tensor_scalar(out=maskf[:], in0=vals[:], scalar1=-1.0e30,
                                scalar2=None, op0=OP.is_ge)
        nc.sync.dma_start(out=out_maskf[:], in_=maskf[:])

        # importance = softmax(max(vals, -1e9)) per row
        timp = small.tile([BL, K], F32)
        nc.vector.tensor_scalar(out=timp[:], in0=vals[:], scalar1=-1.0e9,
                                scalar2=None, op0=OP.max)
        mx = small.tile([BL, 1], F32)
        nc.vector.tensor_reduce(out=mx[:], in_=timp[:], axis=AX.X, op=OP.max)
        negmx = small.tile([BL, 1], F32)
        nc.vector.tensor_scalar(out=negmx[:], in0=mx[:], scalar1=-1.0,
                                scalar2=None, op0=OP.mult)
        ex = small.tile([BL, K], F32)
        nc.scalar.activation(out=ex[:], in_=timp[:], func=AF.Exp,
                             bias=negmx[:, 0:1], scale=1.0)
        sm = small.tile([BL, 1], F32)
        nc.vector.tensor_reduce(out=sm[:], in_=ex[:], axis=AX.X, op=OP.add)
        rs = small.tile([BL, 1], F32)
        nc.vector.reciprocal(out=rs[:], in_=sm[:])
        imp = small.tile([BL, K], F32)
        nc.vector.tensor_scalar(out=imp[:], in0=ex[:], scalar1=rs[:, 0:1],
                                scalar2=None, op0=OP.mult)
        nc.sync.dma_start(out=out_imp[:], in_=imp[:])

        # gather selected tokens: global index = idx + b*2048 into flat latent
        # (indirect DMA requires the source AP to start at offset 0)
        rowoff = const.tile([BL, 1], mybir.dt.int32)
        nc.gpsimd.iota(rowoff[:], pattern=[[0, 1]], base=0, channel_multiplier=N)
        gidx = const.tile([BL, K], mybir.dt.int32)
        nc.vector.tensor_tensor(out=gidx[:], in0=idx_i32[:],
                                in1=rowoff[:, 0:1].to_broadcast([BL, K]),
                                op=OP.add)
        lat_flat = lat.ap().rearrange("b n d -> (b n) d")
        for b in range(BL):
            idxcol = small.tile([K, 1], mybir.dt.int32, tag="idxcol")
            nc.sync.dma_start(out=idxcol[:], in_=gidx[b:b + 1, :])
            gath = work.tile([K, D], F32, tag="gath")
            nc.gpsimd.indirect_dma_start(
                out=gath[:], out_offset=None,
                in_=lat_flat,
                in_offset=bass.IndirectOffsetOnAxis(ap=idxcol[:, 0:1], axis=0))
            nc.sync.dma_start(out=out_tok[b * K:(b + 1) * K, :], in_=gath[:])

    nc.finalize()  # Bacc register allocation + freeze (axon path needs it done)
    return nc


_NC = None


def _get_nc():
    global _NC
    if _NC is None:
        _NC = build_nc()
    return _NC


def make_in_maps(latent, token_mask, Wq, bq, Wk, bk, Ws, bs):
    latent = np.ascontiguousarray(np.asarray(latent, dtype=np.float32))
    mask01 = np.asarray(token_mask).astype(np.float32)
    wqT = np.ascontiguousarray(np.asarray(Wq, dtype=np.float32).T)
    wk = np.ascontiguousarray(np.asarray(Wk, dtype=np.float32))
    ws16 = np.ascontiguousarray(16.0 * np.asarray(Ws, dtype=np.float32))[None, :]
    bqd = np.ascontiguousarray(np.asarray(bq, dtype=np.float32))
    in_maps = []
    for c in range(NCORES):
        sl = slice(c * BL, (c + 1) * BL)
        in_maps.append({
            "latent": np.ascontiguousarray(latent[sl]),
            "mask01": np.ascontiguousarray(mask01[sl]),
            "wqT": wqT, "wk": wk, "ws16": ws16, "bqd": bqd,
        })
    return in_maps


def assemble(results):
    sel_tokens = np.concatenate(
        [r["out_tokens"].reshape(BL, K, D) for r in results], axis=0)
    sel_idx = np.concatenate([r["out_idx"] for r in results], axis=0)
    sel_mask = np.concatenate([r["out_maskf"] for r in results], axis=0) > 0.5
    importance = np.concatenate([r["out_imp"] for r in results], axis=0)
    global_latent = np.concatenate([r["out_glob"] for r in results], axis=0)
    return (sel_tokens.astype(np.float32), sel_mask,
            sel_idx.astype(np.int32), importance.astype(np.float32),
            global_latent.astype(np.float32))


def kernel(latent, token_mask, Wq, bq, Wk, bk, Ws, bs):
    nc = _get_nc()
    in_maps = make_in_maps(latent, token_mask, Wq, bq, Wk, bk, Ws, bs)
    res = run_bass_kernel_spmd(nc, in_maps, list(range(NCORES)))
    return assemble(res.results)
